# revision 1
# baseline (speedup 1.0000x reference)
"""CCAttention (criss-cross attention, no softmax) on 8 TRN2 NeuronCores.

Linearized (energies never materialized):
  out[c,h,w] = g*(sum_q Q[q,h,w]*(M_col[q,c,w]+M_row[q,c,h]) + NEG*V[c,h,w]) + x
  M_col[q,c,w] = sum_h K[q,h,w]V[c,h,w] ;  M_row[q,c,h] = sum_w K[q,h,w]V[c,h,w]
R := x + g*NEG*V = (I + g*NEG*wv)x + g*NEG*bv  -> extra projection, accumulated
directly into the mm2-row PSUM.  gamma folded into the M evict scale.

Layout: one batch at a time per core; W split into halves s=w//64 stacked on
partitions (p = c + 64 s) so every elementwise pass uses all 128 lanes.
Double xbar-transpose permutes the w axis to 2*(w%64)+s on partitions; mm1-row
only contracts over that axis, so the permutation is harmless.

Sharding: data-parallel over B=32 -> 8 cores x 4 batches.
"""
import numpy as np

import concourse.bass as bass
import concourse.bacc as bacc
import concourse.mybir as mybir
from concourse.tile import TileContext
from concourse.bass_utils import run_bass_kernel_spmd

B, C, H, W = 32, 64, 128, 128
HW = H * W
NEG = -1e4
NCORES = 8
BLOC = B // NCORES
F32 = mybir.dt.float32
BF16 = mybir.dt.bfloat16
AF = mybir.ActivationFunctionType
ALU = mybir.AluOpType


def build(nc, gamma):
    x_d = nc.dram_tensor("x", [BLOC, C, H, W], F32, kind="ExternalInput")
    wv_d = nc.dram_tensor("wvt", [128, 64], F32, kind="ExternalInput")
    wqk_d = nc.dram_tensor("wqkt", [128, 16], F32, kind="ExternalInput")
    wtr_d = nc.dram_tensor("wrt", [128, 64], F32, kind="ExternalInput")
    cst_d = nc.dram_tensor("cst", [128, 4], F32, kind="ExternalInput")
    out_d = nc.dram_tensor("out", [BLOC, C, H, W], F32, kind="ExternalOutput")

    with TileContext(nc) as tc:
        with (
            tc.tile_pool(name="wp", bufs=1) as wp,
            tc.tile_pool(name="sb", bufs=1) as sb,
            tc.tile_pool(name="ps", bufs=6, space="PSUM") as pp,
        ):
            wv = wp.tile([128, 64], BF16, tag="wv")
            wqk = wp.tile([128, 16], BF16, tag="wqk")
            wtr = wp.tile([128, 64], BF16, tag="wtr")
            cst = wp.tile([128, 4], F32, tag="cst")
            nc.gpsimd.dma_start(out=wv[:, :], in_=wv_d[:, :], single_packet=True)
            nc.gpsimd.dma_start(out=wqk[:, :], in_=wqk_d[:, :], single_packet=True)
            nc.gpsimd.dma_start(out=wtr[:, :], in_=wtr_d[:, :], single_packet=True)
            nc.sync.dma_start(out=cst[:, :], in_=cst_d[:, :], single_packet=True)

            for b in range(BLOC):
                batch(nc, sb, pp, x_d, out_d, wv, wqk, wtr, cst, b, float(gamma))
    return nc


def batch(nc, sb, pp, x_d, out_d, wv, wqk, wtr, cst, b, g):
    # ---- load x: [c+64s, h*64+wl] bf16 (cast dma) ----
    xH = sb.tile([128, 8192], BF16, tag="xH")
    for s in range(2):
        nc.gpsimd.dma_start(
            out=xH[64 * s : 64 * s + 64, :],
            in_=x_d[b, :, :, 64 * s : 64 * s + 64],
        )

    # ---- projections (V, QK) ----
    Vs = sb.tile([128, 8192], BF16, tag="Vs")
    QK = sb.tile([128, 8192], BF16, tag="QK")
    for ci in range(16):
        sl = slice(512 * ci, 512 * ci + 512)
        psV = pp.tile([128, 512], F32, tag="ps")
        for s in range(2):
            nc.tensor.matmul(
                out=psV[64 * s : 64 * s + 64, :],
                lhsT=wv[64 * s : 64 * s + 64, :],
                rhs=xH[64 * s : 64 * s + 64, sl],
                start=True, stop=True, tile_position=(64 * s, 64 * s),
            )
        nc.scalar.activation(out=Vs[:, sl], in_=psV[:, :], func=AF.Identity,
                             bias=cst[:, 0:1], scale=1.0)
        psQ = pp.tile([128, 512], F32, tag="ps")
        for s in range(2):
            nc.tensor.matmul(
                out=psQ[32 * s : 32 * s + 16, :],
                lhsT=wqk[64 * s : 64 * s + 64, :],
                rhs=xH[64 * s : 64 * s + 64, sl],
                start=True, stop=True, tile_position=(64 * s, 32 * s),
            )
        esc = sb.tile([128, 512], F32, tag="esc")
        nc.scalar.activation(out=esc[:, :], in_=psQ[:, :], func=AF.Exp,
                             bias=cst[:, 1:2], scale=1.0)
        nc.scalar.activation(out=QK[:, sl], in_=esc[:, :], func=AF.Ln,
                             bias=cst[:, 3:4], scale=1.0)

    # ---- xbar transposes ----
    # VTc[h][wl][p0=c+64s]  <- T(Vs)
    VTc = sb.tile([128, 64, 128], BF16, tag="VTc")
    nc.sync.dma_start(out=VTc[:, :, :], in_=Vs[:, :], transpose=True)
    # VTr[2wl+s][c][h]      <- T(VTc)
    VTr = sb.tile([128, 64, 128], BF16, tag="VTr")
    nc.sync.dma_start(out=VTr[:, :, :],
                      in_=VTc[:, :, :].rearrange("h wl p -> h (wl p)"), transpose=True)
    # QTc[h][wl][p0=32s+qk] <- T(QK[0:64])
    QTc = sb.tile([128, 64, 64], BF16, tag="QTc")
    nc.sync.dma_start(out=QTc[:, :, :], in_=QK[0:64, :], transpose=True)
    # QTr[2wl+s][qk(32)][h] <- T(QTc)
    QTr = sb.tile([128, 32, 128], BF16, tag="QTr")
    nc.sync.dma_start(out=QTr[:, :, :],
                      in_=QTc[:, :, :].rearrange("h wl p -> h (wl p)"), transpose=True)

    # ---- mm1-col: M_col[q,c,w] ----
    Msc = sb.tile([128, 8192], BF16, tag="Msc")  # [32s+q, 512*(w//8)+64*(w%8)+c]
    for t in range(16):
        psM = pp.tile([128, 512], F32, tag="ps")
        for dw in range(8):
            w = 8 * t + dw
            s, wl = w // 64, w % 64
            nc.tensor.matmul(
                out=psM[32 * s : 32 * s + 8, 64 * dw : 64 * dw + 64],
                lhsT=QTc[:, wl, 32 * s + 8 : 32 * s + 16],
                rhs=VTc[:, wl, 64 * s : 64 * s + 64],
                start=True, stop=True, tile_position=(0, 32 * s),
            )
        nc.vector.tensor_scalar_mul(Msc[:, 512 * t : 512 * t + 512], psM[:, :], g)

    # ---- mm1-row: M_row[q,c,h] (written to both 32-row blocks) ----
    Msr = sb.tile([128, 8192], BF16, tag="Msr")
    for t in range(16):
        psN = pp.tile([128, 512], F32, tag="ps")
        for dh in range(8):
            h = 8 * t + dh
            for m in range(2):
                nc.tensor.matmul(
                    out=psN[32 * m : 32 * m + 8, 64 * dh : 64 * dh + 64],
                    lhsT=QTr[:, 8:16, h],
                    rhs=VTr[:, :, h],
                    start=True, stop=True, tile_position=(0, 32 * m),
                )
        nc.vector.tensor_scalar_mul(Msr[:, 512 * t : 512 * t + 512], psN[:, :], g)

    # ---- mm2-row + R-projection -> ORs (natural half layout h*64+wl) ----
    ORs = sb.tile([128, 8192], BF16, tag="ORs")
    for t in range(16):
        psR = pp.tile([128, 512], F32, tag="ps")
        for s in range(2):
            nc.tensor.matmul(
                out=psR[64 * s : 64 * s + 64, :],
                lhsT=wtr[64 * s : 64 * s + 64, :],
                rhs=xH[64 * s : 64 * s + 64, 512 * t : 512 * t + 512],
                start=True, stop=False, tile_position=(64 * s, 64 * s),
            )
        for dh in range(8):
            h = 8 * t + dh
            for s in range(2):
                nc.tensor.matmul(
                    out=psR[64 * s : 64 * s + 64, 64 * dh : 64 * dh + 64],
                    lhsT=Msr[32 * s : 32 * s + 8, 512 * t + 64 * dh : 512 * t + 64 * dh + 64],
                    rhs=QK[32 * s : 32 * s + 8, 64 * h : 64 * h + 64],
                    start=False, stop=True, tile_position=(32 * s, 64 * s),
                )
        nc.scalar.activation(out=ORs[:, 512 * t : 512 * t + 512], in_=psR[:, :],
                             func=AF.Identity, bias=cst[:, 2:3], scale=1.0)

    # ---- mm2-col + final merge -> OUT ----
    OUT = sb.tile([128, 8192], BF16, tag="OUT")
    for G in range(16):  # wl groups of 4, both halves per tile
        psC = pp.tile([128, 512], F32, tag="ps")
        for s in range(2):
            for dw in range(4):
                wl = 4 * G + dw
                w = 64 * s + wl
                nc.tensor.matmul(
                    out=psC[64 * s : 64 * s + 64, 128 * dw : 128 * dw + 128],
                    lhsT=Msc[32 * s : 32 * s + 8,
                             512 * (w // 8) + 64 * (w % 8) : 512 * (w // 8) + 64 * (w % 8) + 64],
                    rhs=QK[32 * s : 32 * s + 8, :]
                        .rearrange("q (h wl) -> q wl h", wl=64)[:, wl, :],
                    start=True, stop=True, tile_position=(32 * s, 64 * s),
                )
        oap = OUT[:, :].rearrange("p (h wl) -> p wl h", wl=64)[:, 4 * G : 4 * G + 4, :]
        rap = ORs[:, :].rearrange("p (h wl) -> p wl h", wl=64)[:, 4 * G : 4 * G + 4, :]
        nc.vector.scalar_tensor_tensor(
            out=oap,
            in0=psC[:, :].rearrange("p (a h) -> p a h", h=128), scalar=1.0,
            in1=rap,
            op0=ALU.mult, op1=ALU.add,
        )

    # ---- store (bf16 -> f32 cast dma) ----
    for s in range(2):
        nc.gpsimd.dma_start(
            out=out_d[b, :, :, 64 * s : 64 * s + 64],
            in_=OUT[64 * s : 64 * s + 64, :],
        )


def _prep(wq, bq, wk, bk, wv, bv, g):
    wv_t = np.concatenate([wv.T, wv.T], axis=0).astype(np.float32)            # [128,64]
    wqk1 = np.concatenate([wq, wk], axis=0).T.astype(np.float32)              # [64,16]
    wqk_t = np.concatenate([wqk1, wqk1], axis=0)                              # [128,16]
    wR = (np.eye(C, dtype=np.float32) + g * NEG * wv).T
    wr_t = np.concatenate([wR, wR], axis=0).astype(np.float32)                # [128,64]
    c0 = np.concatenate([bv, bv]).astype(np.float32)
    c1 = np.zeros(128, np.float32)
    for blk in range(4):
        c1[32 * blk : 32 * blk + 8] = bq
        c1[32 * blk + 8 : 32 * blk + 16] = bk
    c2 = np.concatenate([g * NEG * bv, g * NEG * bv]).astype(np.float32)
    cst = np.stack([c0, c1, c2, np.ones(128, np.float32)], axis=1)
    return wv_t, wqk_t, wr_t, cst


def kernel(x, wq, bq, wk, bk, wv, bv, gamma):
    g = float(np.asarray(gamma).reshape(-1)[0])
    wv_t, wqk_t, wr_t, cst = _prep(wq, bq, wk, bk, wv, bv, g)

    nc = bacc.Bacc()
    build(nc, g)
    nc.finalize()

    in_maps = []
    for i in range(NCORES):
        in_maps.append({
            "x": np.ascontiguousarray(x[BLOC * i : BLOC * (i + 1)]).astype(np.float32),
            "wvt": wv_t, "wqkt": wqk_t, "wrt": wr_t, "cst": cst,
        })
    res = run_bass_kernel_spmd(nc, in_maps, core_ids=list(range(NCORES)))
    global LAST_RESULT
    LAST_RESULT = res
    out = np.concatenate([res.results[i]["out"] for i in range(NCORES)], axis=0)
    return out.astype(np.float32)


LAST_RESULT = None

